# revision 1
# baseline (speedup 1.0000x reference)
"""Trainium2 Bass kernel for the gnn_message_passing LoopModel.

Reference computation (per edge e, corners l/r from edge_corner):
    CF[n]    = mean over pairs (n, e') of x[e']          (segment mean)
    out[e]   = relu(W1 @ x[e] + W2 @ CF[l_e] + W3 @ CF[r_e] + W4 @ max_e x)

Distribution over 8 NeuronCores:
  - corner table sharded 32 corners/core (host balances incident-pair load),
    scatter stage = dma_gather of incident x rows + one matmul with a
    host-built scatter matrix (1/count folded in), AllGather of table slices
  - global max: edge-sharded local max + AllReduce(max)
  - conv stage edge-sharded 64 edges/core: dma_gather of left/right corner
    rows + accumulating matmuls (2 edges batched per 128-partition matmul)

HW is padded 784 -> 832 floats so gather rows are 256B-aligned.
"""

import os
import sys
import numpy as np

for _p in ("/opt/trn_rl_repo", "/root/.axon_site/_ro/trn_rl_repo"):
    if os.path.isdir(_p) and _p not in sys.path:
        sys.path.insert(0, _p)

from concourse import bacc, bass, mybir, tile  # noqa: E402
from concourse.bass_utils import run_bass_kernel_spmd  # noqa: E402

N_CORES = 8
E, C, H, W = 512, 64, 28, 28
HW = H * W                      # 784
HWP = 832                       # padded to 13*64 floats (3328B, 256B-aligned)
NC_TOT = 256                    # corner table rows (padded if num_corners < 256)
N_LOC = NC_TOT // N_CORES       # 32 corners per core
E_LOC = E // N_CORES            # 64 edges per core

# matmul input dtype: float32r streams 1 row/cycle (TF32-like precision),
# float32 is exact but 4 rows/cycle.  Overridable for experiments.
MM_DT_NAME = os.environ.get("KERNEL_MM_DT", "float32r")

_PROGRAM_CACHE = {}


# --------------------------------------------------------------------------
# host-side index preparation
# --------------------------------------------------------------------------

def _balance_corners(counts):
    """Assign NC_TOT corners to N_CORES bins, N_LOC corners per bin,
    minimizing the max total incident-pair count per bin.
    Returns (assign[NC_TOT] -> core, loads[N_CORES])."""
    order = np.argsort(-counts, kind="stable")
    loads = np.zeros(N_CORES, dtype=np.int64)
    slots = np.zeros(N_CORES, dtype=np.int64)
    assign = np.full(NC_TOT, -1, dtype=np.int64)
    for c in order:
        cand = [b for b in range(N_CORES) if slots[b] < N_LOC]
        b = min(cand, key=lambda i: (loads[i], slots[i]))
        assign[c] = b
        loads[b] += counts[c]
        slots[b] += 1
    # local swap repair toward equal loads
    target = counts.sum() // N_CORES
    for _ in range(4096):
        hi = int(np.argmax(loads))
        lo = int(np.argmin(loads))
        if loads[hi] <= max(target, 128):
            break
        best = None
        ch = np.where(assign == hi)[0]
        cl = np.where(assign == lo)[0]
        for a in ch:
            for b2 in cl:
                d = counts[a] - counts[b2]
                if 0 < d <= loads[hi] - loads[lo]:
                    if best is None or abs(d - (loads[hi] - target)) < abs(
                        best[2] - (loads[hi] - target)
                    ):
                        best = (a, b2, d)
        if best is None:
            break
        a, b2, d = best
        assign[a], assign[b2] = lo, hi
        loads[hi] -= d
        loads[lo] += d
    return assign, loads


def _wrap_idxs(idx_flat, n_pad):
    """Pack flat gather indices into the dma_gather wrapped layout:
    [128, n_pad//16] int16 with logical index i at [i%16, i//16],
    replicated across the 8 groups of 16 partitions."""
    assert n_pad % 16 == 0
    w = np.zeros((16, n_pad // 16), dtype=np.int16)
    for i, v in enumerate(idx_flat):
        w[i % 16, i // 16] = v
    return np.tile(w, (8, 1))


def _prepare(x, W_agg, corner_edge_pairs, edge_corner, num_corners):
    x = np.asarray(x, dtype=np.float32)
    W_agg = np.asarray(W_agg, dtype=np.float32)
    cep = np.asarray(corner_edge_pairs).astype(np.int64)
    ec = np.asarray(edge_corner).astype(np.int64)
    ncorn = int(num_corners)
    assert x.shape == (E, C, H, W), x.shape
    assert ncorn <= NC_TOT

    # reference semantics: scatter drops out-of-range segments, gathers clamp
    seg = cep[:, 0]
    eid = np.clip(cep[:, 1], 0, E - 1)
    valid = (seg >= 0) & (seg < ncorn)
    seg_v, eid_v = seg[valid], eid[valid]
    ec_cl = np.clip(ec, 0, max(ncorn - 1, 0))

    counts = np.bincount(seg_v, minlength=NC_TOT).astype(np.int64)
    inv_count = 1.0 / np.maximum(counts, 1).astype(np.float64)

    assign, loads = _balance_corners(counts)
    k_chunks = max(1, int(-(-int(loads.max()) // 128)))  # ceil(maxload/128)
    k_pad = 128 * k_chunks

    # permuted corner position in the all-gathered table
    pos = np.zeros(NC_TOT, dtype=np.int64)
    slot_ctr = np.zeros(N_CORES, dtype=np.int64)
    for c in range(NC_TOT):
        b = assign[c]
        pos[c] = b * N_LOC + slot_ctr[b]
        slot_ctr[b] += 1

    # per-core incident pair lists
    pair_eids = [[] for _ in range(N_CORES)]
    pair_local = [[] for _ in range(N_CORES)]   # local corner slot of each pair
    pair_inv = [[] for _ in range(N_CORES)]
    for p in range(len(seg_v)):
        c = int(seg_v[p])
        b = int(assign[c])
        pair_eids[b].append(int(eid_v[p]))
        pair_local[b].append(int(pos[c] - b * N_LOC))
        pair_inv[b].append(inv_count[c])

    # padded x (and per-core slice), fp32, HW -> HWP
    xpad = np.zeros((E, C, HWP), dtype=np.float32)
    xpad[:, :, :HW] = x.reshape(E, C, HW)
    xf = xpad.reshape(E * 16, 4 * HWP)          # 4-channel gather rows

    # block-diagonal weights for 2-edge batched conv matmuls
    wblk = np.zeros((4, 128, 128), dtype=np.float32)
    for t in range(4):
        wt = W_agg[:, t * 64:(t + 1) * 64].T    # [c, o]
        wblk[t, :64, :64] = wt
        wblk[t, 64:, 64:] = wt
    wblk_in = wblk.reshape(512, 128)

    per_core = []
    for b in range(N_CORES):
        k_real = len(pair_eids[b])
        assert k_real <= k_pad
        eids_b = np.zeros(k_pad, dtype=np.int64)
        eids_b[:k_real] = pair_eids[b]
        mc = np.zeros((k_pad, N_LOC), dtype=np.float32)
        for p in range(k_real):
            mc[p, pair_local[b][p]] += pair_inv[b][p]

        # stage-1 gather indices: per K-chunk, 8 instructions of 2 row-slots
        # i = s*128 + p  (s in 0..1), idx = eid*16 + (2*j + s)
        s1_cols = []
        for kc in range(k_chunks):
            epk = eids_b[kc * 128:(kc + 1) * 128]
            for j in range(8):
                flat = np.zeros(256, dtype=np.int64)
                for s in range(2):
                    flat[s * 128:(s + 1) * 128] = epk * 16 + (2 * j + s)
                s1_cols.append(_wrap_idxs(flat.astype(np.int16), 256))
        s1i = np.concatenate(s1_cols, axis=1)   # [128, 16*k_chunks*... ] int16

        # stage-4 gather indices: 8 instructions x 8 edges (4 edge-pairs)
        # i = s*128 + m*64 + c, s = ep*2 + t, idx = pos(corner)*64 + c
        e0 = b * E_LOC
        s4_cols = []
        for binstr in range(8):
            flat = np.zeros(1024, dtype=np.int64)
            for ep in range(4):
                for t in range(2):
                    s = ep * 2 + t
                    for m in range(2):
                        le = binstr * 8 + ep * 2 + m
                        corner = int(ec_cl[e0 + le, t])
                        base = pos[corner] * 64
                        i0 = s * 128 + m * 64
                        flat[i0:i0 + 64] = base + np.arange(64)
            s4_cols.append(_wrap_idxs(flat.astype(np.int16), 1024))
        s4i = np.concatenate(s4_cols, axis=1)   # [128, 512] int16

        per_core.append(dict(
            mc=mc.reshape(k_pad, N_LOC),
            s1i=s1i,
            s4i=s4i,
            xl=xpad[e0:e0 + E_LOC].reshape(E_LOC * C, HWP),
        ))

    return xf, wblk_in, per_core, k_chunks


# --------------------------------------------------------------------------
# device program
# --------------------------------------------------------------------------

def _build_program(k_chunks, mm_dt_name):
    mm_dt = getattr(mybir.dt, mm_dt_name)
    f32 = mybir.dt.float32
    i16 = mybir.dt.int16

    nc = bacc.Bacc("TRN2", target_bir_lowering=False, debug=False,
                   num_devices=N_CORES)

    xf_t = nc.dram_tensor("xf", [E * 16, 4 * HWP], f32, kind="ExternalInput").ap()
    xl_t = nc.dram_tensor("xl", [E_LOC * C, HWP], f32, kind="ExternalInput").ap()
    wb_t = nc.dram_tensor("wb", [512, 128], f32, kind="ExternalInput").ap()
    mc_t = nc.dram_tensor("mc", [128 * k_chunks, N_LOC], f32, kind="ExternalInput").ap()
    s1_t = nc.dram_tensor("s1i", [128, 16 * 8 * k_chunks], i16, kind="ExternalInput").ap()
    s4_t = nc.dram_tensor("s4i", [128, 512], i16, kind="ExternalInput").ap()
    out_t = nc.dram_tensor("out", [E_LOC * C, HWP], f32, kind="ExternalOutput").ap()

    FR = 13312  # free elems per stage-1 gather tile: 4 rows * 3328

    with tile.TileContext(nc) as tc:
        with tc.tile_pool(name="dram", bufs=1, space="DRAM") as dram, \
             tc.tile_pool(name="consts", bufs=1) as consts:
            cfn_slice = dram.tile([N_LOC * C, HWP], f32)
            cfn_full = dram.tile([NC_TOT * C, HWP], f32, addr_space="Shared")
            gmx_in = dram.tile([C, HWP], f32)
            gmx_out = dram.tile([C, HWP], f32, addr_space="Shared")

            # constants
            wtiles = []
            for t in range(4):
                wt = consts.tile([128, 128], mm_dt, tag=f"w{t}")
                nc.sync.dma_start(out=wt[:], in_=wb_t[t * 128:(t + 1) * 128, :].bitcast(mm_dt))
                wtiles.append(wt)
            mctiles = []
            for kc in range(k_chunks):
                mt = consts.tile([128, N_LOC], mm_dt, tag=f"mc{kc}")
                nc.sync.dma_start(out=mt[:], in_=mc_t[kc * 128:(kc + 1) * 128, :].bitcast(mm_dt))
                mctiles.append(mt)
            s1tile = consts.tile([128, 16 * 8 * k_chunks], i16)
            nc.sync.dma_start(out=s1tile[:], in_=s1_t[:])
            s4tile = consts.tile([128, 512], i16)
            nc.sync.dma_start(out=s4tile[:], in_=s4_t[:])

            # ---------------- phase 1: scatter into corner-table slice ----
            with tc.tile_pool(name="p1", bufs=2) as p1, \
                 tc.tile_pool(name="p1s", bufs=2) as p1s, \
                 tc.tile_pool(name="psum1", bufs=4, space="PSUM") as psum1:
                for j in range(8):
                    stg = p1s.tile([N_LOC, 2 * 3328], f32, tag="stg")
                    for kc in range(k_chunks):
                        gt = p1.tile([128, 2 * 3328], mm_dt, tag="gt")
                        nc.gpsimd.dma_gather(
                            gt[:].rearrange("p (s d) -> p s d", d=3328),
                            xf_t[:].bitcast(mm_dt),
                            s1tile[:, (kc * 8 + j) * 16:(kc * 8 + j) * 16 + 16],
                            num_idxs=256, num_idxs_reg=256, elem_size=3328,
                        )
                        for fc in range(16):
                            ps = psum1.tile([N_LOC, 416], f32, space="PSUM", tag="ps1")
                            nc.tensor.matmul(
                                out=ps[:],
                                lhsT=mctiles[kc][:],
                                rhs=gt[:, fc * 416:(fc + 1) * 416],
                                start=True, stop=True,
                            )
                            if k_chunks == 1:
                                nc.vector.tensor_copy(
                                    out=stg[:, fc * 416:(fc + 1) * 416], in_=ps[:])
                            else:
                                if kc == 0:
                                    nc.vector.tensor_copy(
                                        out=stg[:, fc * 416:(fc + 1) * 416], in_=ps[:])
                                else:
                                    nc.vector.tensor_tensor(
                                        out=stg[:, fc * 416:(fc + 1) * 416],
                                        in0=stg[:, fc * 416:(fc + 1) * 416],
                                        in1=ps[:], op=mybir.AluOpType.add)
                    # staging -> DRAM slice rows m*64 + 8j .. +8
                    nc.sync.dma_start(
                        out=cfn_slice[:].rearrange("(m c) w -> m c w", c=C)[:, 8 * j:8 * j + 8, :],
                        in_=stg[:].rearrange("m (c w) -> m c w", w=HWP),
                    )

            # ---------------- phase 2: global max -------------------------
            xtiles = []
            with tc.tile_pool(name="xkeep", bufs=E_LOC // 2) as xkeep, \
                 tc.tile_pool(name="p2", bufs=2) as p2:
                mx = p2.tile([128, HWP], f32, tag="mx")
                for j in range(E_LOC // 2):
                    xt = xkeep.tile([128, HWP], mm_dt, tag="xt")
                    nc.sync.dma_start(out=xt[:], in_=xl_t[128 * j:128 * (j + 1), :].bitcast(mm_dt))
                    xtiles.append(xt)
                    if j == 0:
                        nc.vector.tensor_copy(out=mx[:], in_=xt[:].bitcast(f32))
                    else:
                        nc.vector.tensor_tensor(out=mx[:], in0=mx[:],
                                                in1=xt[:].bitcast(f32),
                                                op=mybir.AluOpType.max)
                half = p2.tile([64, HWP], f32, tag="half")
                nc.sync.dma_start(out=half[:], in_=mx[64:128, :])
                nc.vector.tensor_tensor(out=mx[0:64, :], in0=mx[0:64, :],
                                        in1=half[:], op=mybir.AluOpType.max)
                nc.sync.dma_start(out=gmx_in[:], in_=mx[0:64, :])
                nc.gpsimd.collective_compute(
                    "AllReduce", mybir.AluOpType.max,
                    replica_groups=[list(range(N_CORES))],
                    ins=[gmx_in.opt()], outs=[gmx_out.opt()],
                )
                gm2 = consts.tile([128, HWP], mm_dt, tag="gm2")
                nc.sync.dma_start(out=gm2[0:64, :], in_=gmx_out[:].bitcast(mm_dt))
                nc.sync.dma_start(out=gm2[64:128, :], in_=gmx_out[:].bitcast(mm_dt))

                # ---------------- phase 3: allgather table ----------------
                nc.gpsimd.collective_compute(
                    "AllGather", mybir.AluOpType.bypass,
                    replica_groups=[list(range(N_CORES))],
                    ins=[cfn_slice.opt()], outs=[cfn_full.opt()],
                )

                # ---------------- phase 4: conv ---------------------------
                with tc.tile_pool(name="p4", bufs=2) as p4, \
                     tc.tile_pool(name="p4o", bufs=3) as p4o, \
                     tc.tile_pool(name="psum4", bufs=4, space="PSUM") as psum4:
                    for binstr in range(8):
                        lrt = p4.tile([128, 8 * HWP], mm_dt, tag="lrt")
                        nc.gpsimd.dma_gather(
                            lrt[:].rearrange("p (s d) -> p s d", d=HWP),
                            cfn_full[:].bitcast(mm_dt),
                            s4tile[:, binstr * 64:(binstr + 1) * 64],
                            num_idxs=1024, num_idxs_reg=1024, elem_size=HWP,
                        )
                        for ep in range(4):
                            xt = xtiles[binstr * 4 + ep]
                            ot = p4o.tile([128, HWP], f32, tag="ot")
                            for hh in range(2):
                                sl = slice(hh * 416, (hh + 1) * 416)
                                ps = psum4.tile([128, 416], f32, space="PSUM", tag="ps4")
                                nc.tensor.matmul(out=ps[:], lhsT=wtiles[0][:],
                                                 rhs=xt[:, sl], start=True, stop=False)
                                nc.tensor.matmul(out=ps[:], lhsT=wtiles[1][:],
                                                 rhs=lrt[:, (ep * 2) * HWP + hh * 416:
                                                         (ep * 2) * HWP + hh * 416 + 416],
                                                 start=False, stop=False)
                                nc.tensor.matmul(out=ps[:], lhsT=wtiles[2][:],
                                                 rhs=lrt[:, (ep * 2 + 1) * HWP + hh * 416:
                                                         (ep * 2 + 1) * HWP + hh * 416 + 416],
                                                 start=False, stop=False)
                                nc.tensor.matmul(out=ps[:], lhsT=wtiles[3][:],
                                                 rhs=gm2[:, sl], start=False, stop=True)
                                nc.scalar.activation(ot[:, sl], ps[:],
                                                     mybir.ActivationFunctionType.Relu)
                            nc.sync.dma_start(
                                out=out_t[(binstr * 8 + ep * 2) * 64:
                                          (binstr * 8 + ep * 2) * 64 + 128, :],
                                in_=ot[:],
                            )

    nc.compile()
    return nc


# --------------------------------------------------------------------------
# entry point
# --------------------------------------------------------------------------

def kernel(x, W_agg, corner_edge_pairs, edge_corner, num_corners):
    xf, wblk_in, per_core, k_chunks = _prepare(
        x, W_agg, corner_edge_pairs, edge_corner, num_corners)

    key = (k_chunks, MM_DT_NAME)
    if key not in _PROGRAM_CACHE:
        _PROGRAM_CACHE[key] = _build_program(k_chunks, MM_DT_NAME)
    nc = _PROGRAM_CACHE[key]

    in_maps = []
    for b in range(N_CORES):
        pc = per_core[b]
        in_maps.append({
            "xf": xf,
            "xl": pc["xl"],
            "wb": wblk_in,
            "mc": pc["mc"],
            "s1i": pc["s1i"],
            "s4i": pc["s4i"],
        })

    res = run_bass_kernel_spmd(nc, in_maps, list(range(N_CORES)))

    out = np.empty((E, C, H, W), dtype=np.float32)
    for b in range(N_CORES):
        ob = res.results[b]["out"].reshape(E_LOC, C, HWP)
        out[b * E_LOC:(b + 1) * E_LOC] = ob[:, :, :HW].reshape(E_LOC, C, H, W)
    return out


# expose for test harness profiling
def _run_profiled(x, W_agg, corner_edge_pairs, edge_corner, num_corners,
                  trace=True):
    xf, wblk_in, per_core, k_chunks = _prepare(
        x, W_agg, corner_edge_pairs, edge_corner, num_corners)
    key = (k_chunks, MM_DT_NAME)
    if key not in _PROGRAM_CACHE:
        _PROGRAM_CACHE[key] = _build_program(k_chunks, MM_DT_NAME)
    nc = _PROGRAM_CACHE[key]
    in_maps = [{
        "xf": xf, "xl": pc["xl"], "wb": wblk_in,
        "mc": pc["mc"], "s1i": pc["s1i"], "s4i": pc["s4i"],
    } for pc in per_core]
    res = run_bass_kernel_spmd(nc, in_maps, list(range(N_CORES)),
                               trace=trace, trace_cores=list(range(N_CORES)))
    out = np.empty((E, C, H, W), dtype=np.float32)
    for b in range(N_CORES):
        ob = res.results[b]["out"].reshape(E_LOC, C, HWP)
        out[b * E_LOC:(b + 1) * E_LOC] = ob[:, :, :HW].reshape(E_LOC, C, H, W)
    return out, res



# revision 3
# speedup vs baseline: 1.5638x; 1.5638x over previous
"""Trainium2 Bass kernel for the gnn_message_passing LoopModel (bf16 pipeline).

Reference computation (per edge e, corners l/r from edge_corner):
    CF[n]    = mean over pairs (n, e') of x[e']          (segment mean)
    out[e]   = relu(W1 @ x[e] + W2 @ CF[l_e] + W3 @ CF[r_e] + W4 @ max_e x)

Distribution over 8 NeuronCores (all data-plane tensors bf16):
  - corner table sharded 32 corners/core (host balances incident-pair load);
    scatter stage = dma_gather of incident x rows (8-channel elems, unpadded
    784-wide => 12544B, 256B-aligned) + matmuls with a host-built scatter
    matrix (1/count folded in), 4 channel-groups packed per 128-partition
    PSUM tile.
  - per-core local max rows are appended to the table slice so ONE AllGather
    moves both the corner table and the 8 local maxes (no separate
    AllReduce); each core max-reduces the 8 gathered max blocks on DVE.
  - conv stage edge-sharded 64 edges/core: dma_gather of left/right corner
    rows (896-elem padded, 1792B) + 4 accumulating matmuls per PSUM tile
    (2 edges batched per 128-partition matmul via block-diagonal weights).
"""

import os
import sys
import numpy as np
import ml_dtypes

for _p in ("/opt/trn_rl_repo", "/root/.axon_site/_ro/trn_rl_repo"):
    if os.path.isdir(_p) and _p not in sys.path:
        sys.path.insert(0, _p)

from concourse import bacc, bass, mybir, tile  # noqa: E402
from concourse.bass_utils import run_bass_kernel_spmd  # noqa: E402

BF16 = ml_dtypes.bfloat16

N_CORES = 8
E, C, H, W = 512, 64, 28, 28
HW = H * W                      # 784
HWP = 896                       # corner-table row pad: 896*2B = 1792B = 7*256
NC_TOT = 256                    # corner table rows (padded if num_corners < 256)
N_LOC = NC_TOT // N_CORES       # 32 corners per core
E_LOC = E // N_CORES            # 64 edges per core
ROWS_LOC = N_LOC * C + C        # 2048 table rows + 64 max rows = 2112
GROW = 6272                     # phase-1 gather elem: 8 channels * 784
NHALF = HW // 2                 # 392-wide matmul chunks

_PROGRAM_CACHE = {}


# --------------------------------------------------------------------------
# host-side index preparation
# --------------------------------------------------------------------------

def _balance_corners(counts):
    """Assign NC_TOT corners to N_CORES bins, N_LOC corners per bin,
    minimizing the max total incident-pair count per bin."""
    order = np.argsort(-counts, kind="stable")
    loads = np.zeros(N_CORES, dtype=np.int64)
    slots = np.zeros(N_CORES, dtype=np.int64)
    assign = np.full(NC_TOT, -1, dtype=np.int64)
    for c in order:
        cand = [b for b in range(N_CORES) if slots[b] < N_LOC]
        b = min(cand, key=lambda i: (loads[i], slots[i]))
        assign[c] = b
        loads[b] += counts[c]
        slots[b] += 1
    target = counts.sum() // N_CORES
    for _ in range(4096):
        hi = int(np.argmax(loads))
        lo = int(np.argmin(loads))
        if loads[hi] <= max(target, 128):
            break
        best = None
        ch = np.where(assign == hi)[0]
        cl = np.where(assign == lo)[0]
        for a in ch:
            for b2 in cl:
                d = counts[a] - counts[b2]
                if 0 < d <= loads[hi] - loads[lo]:
                    if best is None or abs(d - (loads[hi] - target)) < abs(
                        best[2] - (loads[hi] - target)
                    ):
                        best = (a, b2, d)
        if best is None:
            break
        a, b2, d = best
        assign[a], assign[b2] = lo, hi
        loads[hi] -= d
        loads[lo] += d
    return assign, loads


def _wrap_idxs(idx_flat, n_pad):
    """Pack flat gather indices into the dma_gather wrapped layout:
    [128, n_pad//16] int16 with logical index i at [i%16, i//16],
    replicated across the 8 groups of 16 partitions."""
    assert n_pad % 16 == 0
    w = np.zeros((16, n_pad // 16), dtype=np.int16)
    for i, v in enumerate(idx_flat):
        w[i % 16, i // 16] = v
    return np.tile(w, (8, 1))


def _prepare(x, W_agg, corner_edge_pairs, edge_corner, num_corners):
    x = np.asarray(x, dtype=np.float32)
    W_agg = np.asarray(W_agg, dtype=np.float32)
    cep = np.asarray(corner_edge_pairs).astype(np.int64)
    ec = np.asarray(edge_corner).astype(np.int64)
    ncorn = int(num_corners)
    assert x.shape == (E, C, H, W), x.shape
    assert ncorn <= NC_TOT

    # reference semantics: scatter drops out-of-range segments, gathers clamp
    seg = cep[:, 0]
    eid = np.clip(cep[:, 1], 0, E - 1)
    valid = (seg >= 0) & (seg < ncorn)
    seg_v, eid_v = seg[valid], eid[valid]
    ec_cl = np.clip(ec, 0, max(ncorn - 1, 0))

    counts = np.bincount(seg_v, minlength=NC_TOT).astype(np.int64)
    inv_count = 1.0 / np.maximum(counts, 1).astype(np.float64)

    assign, loads = _balance_corners(counts)
    k_chunks = max(1, int(-(-int(loads.max()) // 128)))  # ceil(maxload/128)
    k_pad = 128 * k_chunks

    # permuted corner position in the all-gathered table
    pos = np.zeros(NC_TOT, dtype=np.int64)
    slot_ctr = np.zeros(N_CORES, dtype=np.int64)
    for c in range(NC_TOT):
        b = assign[c]
        pos[c] = b * N_LOC + slot_ctr[b]
        slot_ctr[b] += 1

    # per-core incident pair lists
    pair_eids = [[] for _ in range(N_CORES)]
    pair_local = [[] for _ in range(N_CORES)]
    pair_inv = [[] for _ in range(N_CORES)]
    for p in range(len(seg_v)):
        c = int(seg_v[p])
        b = int(assign[c])
        pair_eids[b].append(int(eid_v[p]))
        pair_local[b].append(int(pos[c] - b * N_LOC))
        pair_inv[b].append(inv_count[c])

    xbf = x.reshape(E, C * HW).astype(BF16)
    xf = xbf.reshape(E * 8, GROW)               # 8-channel gather rows

    # block-diagonal weights for 2-edge batched conv matmuls
    wblk = np.zeros((4, 128, 128), dtype=BF16)
    for t in range(4):
        wt = W_agg[:, t * 64:(t + 1) * 64].T.astype(BF16)    # [c, o]
        wblk[t, :64, :64] = wt
        wblk[t, 64:, 64:] = wt
    wblk_in = wblk.reshape(512, 128)

    per_core = []
    for b in range(N_CORES):
        e0 = b * E_LOC
        k_real = len(pair_eids[b])
        assert k_real <= k_pad
        eids_b = np.zeros(k_pad, dtype=np.int64)
        eids_b[:k_real] = pair_eids[b]
        mc = np.zeros((k_pad, N_LOC), dtype=np.float32)
        for p in range(k_real):
            mc[p, pair_local[b][p]] += pair_inv[b][p]

        # stage-1 gather indices: per (kc, j): 128 idxs = eid*8 + j
        s1_cols = []
        for kc in range(k_chunks):
            epk = eids_b[kc * 128:(kc + 1) * 128]
            for j in range(8):
                s1_cols.append(_wrap_idxs((epk * 8 + j).astype(np.int16), 128))
        s1i = np.concatenate(s1_cols, axis=1)   # [128, k_chunks*64] int16

        # stage-4 gather indices: 8 instructions x 1024 idxs
        # flat[s*128 + m*64 + c] = row of (corner of edge el=m*32+binstr*4+ep,
        # side t), s = ep*2 + t; row = rank*ROWS_LOC + slot*64 + c
        s4_cols = []
        for binstr in range(8):
            flat = np.zeros(1024, dtype=np.int64)
            for ep in range(4):
                for t in range(2):
                    s = ep * 2 + t
                    for m in range(2):
                        el = m * 32 + binstr * 4 + ep
                        corner = int(ec_cl[e0 + el, t])
                        r = int(assign[corner])
                        slot = int(pos[corner] - r * N_LOC)
                        base = r * ROWS_LOC + slot * 64
                        i0 = s * 128 + m * 64
                        flat[i0:i0 + 64] = base + np.arange(64)
            s4_cols.append(_wrap_idxs(flat.astype(np.int16), 1024))
        s4i = np.concatenate(s4_cols, axis=1)   # [128, 512] int16

        # local x, SBUF layout: [p = m*64+c, e32*784 + w]
        xl = (xbf[e0:e0 + E_LOC]
              .reshape(2, 32, C, HW)            # (m, e32, c, w)
              .transpose(0, 2, 1, 3)            # (m, c, e32, w)
              .reshape(128, 32 * HW))
        xl = np.ascontiguousarray(xl)

        per_core.append(dict(
            mc=mc.astype(BF16),
            s1i=s1i,
            s4i=s4i,
            xl=xl,
        ))

    return xf, wblk_in, per_core, k_chunks


# --------------------------------------------------------------------------
# device program
# --------------------------------------------------------------------------

def _build_program(k_chunks):
    bf = mybir.dt.bfloat16
    f32 = mybir.dt.float32
    i16 = mybir.dt.int16

    nc = bacc.Bacc("TRN2", target_bir_lowering=False, debug=False,
                   num_devices=N_CORES)

    xf_t = nc.dram_tensor("xf", [E * 8, GROW], bf, kind="ExternalInput").ap()
    xl_t = nc.dram_tensor("xl", [128, 32 * HW], bf, kind="ExternalInput").ap()
    wb_t = nc.dram_tensor("wb", [512, 128], bf, kind="ExternalInput").ap()
    mc_t = nc.dram_tensor("mc", [128 * k_chunks, N_LOC], bf, kind="ExternalInput").ap()
    s1_t = nc.dram_tensor("s1i", [128, k_chunks * 64], i16, kind="ExternalInput").ap()
    s4_t = nc.dram_tensor("s4i", [128, 512], i16, kind="ExternalInput").ap()
    out_t = nc.dram_tensor("out", [128, 32 * HW], bf, kind="ExternalOutput").ap()

    with tile.TileContext(nc) as tc:
        with tc.tile_pool(name="dram", bufs=1, space="DRAM") as dram, \
             tc.tile_pool(name="consts", bufs=1) as consts:
            agin = dram.tile([ROWS_LOC, HWP], bf)
            agout = dram.tile([N_CORES * ROWS_LOC, HWP], bf, addr_space="Shared")

            # constants
            wtiles = []
            for t in range(4):
                wt = consts.tile([128, 128], bf, tag=f"w{t}")
                nc.sync.dma_start(out=wt[:], in_=wb_t[t * 128:(t + 1) * 128, :])
                wtiles.append(wt)
            mctiles = []
            for kc in range(k_chunks):
                mt = consts.tile([128, N_LOC], bf, tag=f"mc{kc}")
                nc.sync.dma_start(out=mt[:], in_=mc_t[kc * 128:(kc + 1) * 128, :])
                mctiles.append(mt)
            s1tile = consts.tile([128, k_chunks * 64], i16)
            nc.sync.dma_start(out=s1tile[:], in_=s1_t[:])
            s4tile = consts.tile([128, 512], i16)
            nc.sync.dma_start(out=s4tile[:], in_=s4_t[:])
            gm2 = consts.tile([128, HW], bf, tag="gm2")

            # local x, kept in SBUF through phase 4
            xkeep = consts.tile([128, 32 * HW], bf, tag="xkeep")
            nc.sync.dma_start(out=xkeep[:], in_=xl_t[:])

            # ---------------- phase 1: scatter into corner-table slice ----
            with tc.tile_pool(name="p1", bufs=4 * k_chunks + 1) as p1, \
                 tc.tile_pool(name="p1s", bufs=2) as p1s, \
                 tc.tile_pool(name="psum1", bufs=4, space="PSUM") as psum1:
                gts = {}
                for kc in range(k_chunks):
                    for j in range(8):
                        gt = p1.tile([128, GROW], bf, tag="gt")
                        nc.gpsimd.dma_gather(
                            gt[:].rearrange("p (s d) -> p s d", d=GROW),
                            xf_t[:],
                            s1tile[:, (kc * 8 + j) * 8:(kc * 8 + j) * 8 + 8],
                            num_idxs=128, num_idxs_reg=128, elem_size=GROW,
                        )
                        gts[(kc, j)] = gt
                for q2 in range(2):
                    stg = p1s.tile([128, 8 * HWP], bf, tag="stg")
                    for n in range(16):
                        c8, half = n // 2, n % 2
                        ps = psum1.tile([128, NHALF], f32, space="PSUM", tag="ps1")
                        for qq in range(4):
                            j = q2 * 4 + qq
                            for kc in range(k_chunks):
                                nc.tensor.matmul(
                                    out=ps[qq * 32:(qq + 1) * 32, :],
                                    lhsT=mctiles[kc][:],
                                    rhs=gts[(kc, j)][:, n * NHALF:(n + 1) * NHALF],
                                    start=(kc == 0), stop=(kc == k_chunks - 1),
                                    tile_position=(0, qq * 32),
                                )
                        nc.vector.tensor_copy(
                            out=stg[:, c8 * HWP + half * NHALF:
                                    c8 * HWP + half * NHALF + NHALF],
                            in_=ps[:])
                    # staging -> DRAM slice rows m*64 + j*8 + c8, one DMA per j
                    for qq in range(4):
                        j = q2 * 4 + qq
                        nc.sync.dma_start(
                            out=agin[0:N_LOC * C, :].rearrange(
                                "(m j c8) w -> j m (c8 w)", m=N_LOC, j=8, c8=8)[j],
                            in_=stg[qq * 32:(qq + 1) * 32, :],
                        )

            # ---------------- phase 2: local max --------------------------
            with tc.tile_pool(name="p2", bufs=1) as p2:
                scr = p2.tile([128, 16 * HW], bf, tag="scr")
                nc.vector.tensor_tensor(out=scr[:], in0=xkeep[:, 0:16 * HW],
                                        in1=xkeep[:, 16 * HW:32 * HW],
                                        op=mybir.AluOpType.max)
                wdt = 8 * HW
                while wdt >= HW:
                    nc.vector.tensor_tensor(out=scr[:, 0:wdt], in0=scr[:, 0:wdt],
                                            in1=scr[:, wdt:2 * wdt],
                                            op=mybir.AluOpType.max)
                    wdt //= 2
                half = p2.tile([64, HW], bf, tag="half")
                nc.sync.dma_start(out=half[:], in_=scr[64:128, 0:HW])
                nc.vector.tensor_tensor(out=scr[0:64, 0:HW], in0=scr[0:64, 0:HW],
                                        in1=half[:], op=mybir.AluOpType.max)
                # local max rows -> table tail
                nc.sync.dma_start(out=agin[N_LOC * C:ROWS_LOC, 0:HW],
                                  in_=scr[0:64, 0:HW])

            # ---------------- phase 3: one allgather ----------------------
            nc.gpsimd.collective_compute(
                "AllGather", mybir.AluOpType.bypass,
                replica_groups=[list(range(N_CORES))],
                ins=[agin.opt()], outs=[agout.opt()],
            )

            # global max = DVE max over the 8 gathered local-max blocks
            with tc.tile_pool(name="pgm", bufs=3) as pgm:
                for r in range(N_CORES):
                    gmr = pgm.tile([64, HW], bf, tag="gmr")
                    nc.sync.dma_start(
                        out=gmr[:],
                        in_=agout[r * ROWS_LOC + N_LOC * C:
                                  r * ROWS_LOC + ROWS_LOC, 0:HW])
                    if r == 0:
                        nc.vector.tensor_copy(out=gm2[0:64, :], in_=gmr[:])
                    else:
                        nc.vector.tensor_tensor(out=gm2[0:64, :],
                                                in0=gm2[0:64, :], in1=gmr[:],
                                                op=mybir.AluOpType.max)
                nc.sync.dma_start(out=gm2[64:128, :], in_=gm2[0:64, :])

            # ---------------- phase 4: conv -------------------------------
            with tc.tile_pool(name="p4", bufs=2) as p4, \
                 tc.tile_pool(name="p4o", bufs=3) as p4o, \
                 tc.tile_pool(name="psum4", bufs=4, space="PSUM") as psum4:
                for binstr in range(8):
                    lrt = p4.tile([128, 8 * HWP], bf, tag="lrt")
                    nc.gpsimd.dma_gather(
                        lrt[:].rearrange("p (s d) -> p s d", d=HWP),
                        agout[:],
                        s4tile[:, binstr * 64:(binstr + 1) * 64],
                        num_idxs=1024, num_idxs_reg=1024, elem_size=HWP,
                    )
                    for ep in range(4):
                        e32 = binstr * 4 + ep
                        ot = p4o.tile([128, HW], bf, tag="ot")
                        for hh in range(2):
                            sl = slice(hh * NHALF, (hh + 1) * NHALF)
                            ps = psum4.tile([128, NHALF], f32, space="PSUM", tag="ps4")
                            nc.tensor.matmul(out=ps[:], lhsT=wtiles[0][:],
                                             rhs=xkeep[:, e32 * HW + hh * NHALF:
                                                       e32 * HW + hh * NHALF + NHALF],
                                             start=True, stop=False)
                            nc.tensor.matmul(out=ps[:], lhsT=wtiles[1][:],
                                             rhs=lrt[:, (ep * 2) * HWP + hh * NHALF:
                                                     (ep * 2) * HWP + hh * NHALF + NHALF],
                                             start=False, stop=False)
                            nc.tensor.matmul(out=ps[:], lhsT=wtiles[2][:],
                                             rhs=lrt[:, (ep * 2 + 1) * HWP + hh * NHALF:
                                                     (ep * 2 + 1) * HWP + hh * NHALF + NHALF],
                                             start=False, stop=False)
                            nc.tensor.matmul(out=ps[:], lhsT=wtiles[3][:],
                                             rhs=gm2[:, sl], start=False, stop=True)
                            nc.scalar.activation(ot[:, sl], ps[:],
                                                 mybir.ActivationFunctionType.Relu)
                        nc.sync.dma_start(
                            out=out_t[:, e32 * HW:(e32 + 1) * HW],
                            in_=ot[:],
                        )

    nc.compile()
    return nc


# --------------------------------------------------------------------------
# entry point
# --------------------------------------------------------------------------

def _unpack_out(ob):
    """[128, 32*784] bf16 (p = m*64+c, e32*784+w) -> [64, 64, 28, 28] f32."""
    o = np.asarray(ob, dtype=np.float32).reshape(2, C, 32, HW)
    return o.transpose(0, 2, 1, 3).reshape(E_LOC, C, H, W)


def _run(x, W_agg, corner_edge_pairs, edge_corner, num_corners,
         trace=False):
    xf, wblk_in, per_core, k_chunks = _prepare(
        x, W_agg, corner_edge_pairs, edge_corner, num_corners)

    if k_chunks not in _PROGRAM_CACHE:
        _PROGRAM_CACHE[k_chunks] = _build_program(k_chunks)
    nc = _PROGRAM_CACHE[k_chunks]

    in_maps = [{
        "xf": xf, "xl": pc["xl"], "wb": wblk_in,
        "mc": pc["mc"], "s1i": pc["s1i"], "s4i": pc["s4i"],
    } for pc in per_core]

    kwargs = dict(trace=trace)
    if trace:
        kwargs["trace_cores"] = list(range(N_CORES))
    res = run_bass_kernel_spmd(nc, in_maps, list(range(N_CORES)), **kwargs)

    out = np.empty((E, C, H, W), dtype=np.float32)
    for b in range(N_CORES):
        out[b * E_LOC:(b + 1) * E_LOC] = _unpack_out(res.results[b]["out"])
    return out, res


def kernel(x, W_agg, corner_edge_pairs, edge_corner, num_corners):
    out, _ = _run(x, W_agg, corner_edge_pairs, edge_corner, num_corners,
                  trace=False)
    return out


# expose for test harness profiling
def _run_profiled(x, W_agg, corner_edge_pairs, edge_corner, num_corners,
                  trace=True):
    return _run(x, W_agg, corner_edge_pairs, edge_corner, num_corners,
                trace=trace)
